# revision 6
# baseline (speedup 1.0000x reference)
"""Trainium2 Bass kernel for the ClusterForecasting model.

Strategy: data-parallel over batch B=8 across 8 NeuronCores (one batch
element per core). Each core runs embedding -> 2 transformer layers ->
autoencoder for its batch element, producing the bottleneck codes `low`
(32 points per core) and a partial L1 reconstruction-loss sum. The tiny
cluster stage (cdist/argmin/segment reductions over 256 points, scalar
losses) is finished on host in float32 numpy.

Device layout is "transposed-resident": the residual stream lives as
zT [D, S*T] = 2 x [128, 512] SBUF tiles. All matmuls contract over the
partition dim; LayerNorm stats use all-ones matmuls (partition-dim
reduction with broadcast built in); softmax is computed without max
subtraction (scores are O(1) here) with per-query denominators obtained
from band-ones matmuls and inverted via exp(-ln(d)) on the scalar engine
so only one ACT table set is ever loaded.
"""

import math

import numpy as np

import concourse.bass as bass
import concourse.mybir as mybir
import concourse.tile as tile
from concourse import bacc
from concourse.bass_utils import run_bass_kernel_spmd

F32 = mybir.dt.float32
AF = mybir.ActivationFunctionType
ALU = mybir.AluOpType

# Problem dims (hardcoded per contest contract)
B, S, T, F = 8, 32, 16, 8
D, H, L, FF, K = 256, 8, 2, 1024, 5
NSEQ = S * T          # 512
DH = D // H           # 32
DT = D // 128         # 2 d-tiles
FT = FF // 128        # 8 ff-tiles
ENC_IN = T * D        # 4096
SCALE = 1.0 / math.sqrt(DH)
EPS = 1e-5

_CACHE = {}


def _build_program():
    nc = bacc.Bacc("TRN2", target_bir_lowering=False, debug=False)

    dr = {}

    def din(name, shape):
        dr[name] = nc.dram_tensor(name, list(shape), F32, kind="ExternalInput")
        return dr[name]

    din("xt", (F + 1, NSEQ))            # x_seg[b]^T with ones row appended
    din("embw", (F + 1, D))             # emb_W with emb_b row appended
    din("wq", (L, D, D))
    din("wk", (L, D, D))
    din("wv", (L, D, D))
    din("wo", (L, D, D))
    din("w1", (L, D, FF))
    din("b1r", (L, 128, FT))
    din("w2", (L, FF, D))
    din("b2r", (L, 128, DT))
    din("g1r", (L, 128, DT))
    din("be1r", (L, 128, DT))
    din("g2r", (L, 128, DT))
    din("be2r", (L, 128, DT))
    din("encw1", (ENC_IN, 128))
    din("encb1", (128, 1))
    din("encw2", (128, 2))
    din("encb2n", (2, 1))               # negated enc_b2
    din("decw1", (2, 128))
    din("decb1", (128, 1))
    din("decw2", (128, ENC_IN))
    din("decb2bc", (2, 128, 512))       # dec_b2 broadcast per parity

    out_low = nc.dram_tensor("lowt", [2, S], F32, kind="ExternalOutput")
    out_l1 = nc.dram_tensor("l1p", [128, 2], F32, kind="ExternalOutput")

    with tile.TileContext(nc) as tc:
        with (
            tc.tile_pool(name="const", bufs=1) as const,
            tc.tile_pool(name="act", bufs=2) as act,
            tc.tile_pool(name="psS", bufs=2, space="PSUM") as psS,
            tc.tile_pool(name="psO", bufs=1, space="PSUM") as psO,
            tc.tile_pool(name="psD", bufs=1, space="PSUM") as psD,
            tc.tile_pool(name="psG", bufs=2, space="PSUM") as psG,
        ):
            def cload(name, shape, src):
                t = const.tile(list(shape), F32, name=name)
                nc.sync.dma_start(t[:], src)
                return t

            # ---- constants in SBUF ----
            xt_sb = cload("xt_sb", (F + 1, NSEQ), dr["xt"][:, :])
            embw_sb = cload("embw_sb", (F + 1, D), dr["embw"][:, :])
            wq_sb, wk_sb, wv_sb, wo_sb, w1_sb, w2_sb = [], [], [], [], [], []
            b1_sb, b2_sb, g1_sb, be1_sb, g2_sb, be2_sb = [], [], [], [], [], []
            for l in range(L):
                wq_sb.append([cload(f"wq{l}_{kt}", (128, D), dr["wq"][l, kt * 128:(kt + 1) * 128, :]) for kt in range(DT)])
                wk_sb.append([cload(f"wk{l}_{kt}", (128, D), dr["wk"][l, kt * 128:(kt + 1) * 128, :]) for kt in range(DT)])
                wv_sb.append([cload(f"wv{l}_{kt}", (128, D), dr["wv"][l, kt * 128:(kt + 1) * 128, :]) for kt in range(DT)])
                wo_sb.append([cload(f"wo{l}_{kt}", (128, D), dr["wo"][l, kt * 128:(kt + 1) * 128, :]) for kt in range(DT)])
                w1_sb.append([cload(f"w1{l}_{kt}", (128, FF), dr["w1"][l, kt * 128:(kt + 1) * 128, :]) for kt in range(DT)])
                w2_sb.append([cload(f"w2{l}_{ft}", (128, D), dr["w2"][l, ft * 128:(ft + 1) * 128, :]) for ft in range(FT)])
                b1_sb.append(cload(f"b1_{l}", (128, FT), dr["b1r"][l]))
                b2_sb.append(cload(f"b2_{l}", (128, DT), dr["b2r"][l]))
                g1_sb.append(cload(f"g1_{l}", (128, DT), dr["g1r"][l]))
                be1_sb.append(cload(f"be1_{l}", (128, DT), dr["be1r"][l]))
                g2_sb.append(cload(f"g2_{l}", (128, DT), dr["g2r"][l]))
                be2_sb.append(cload(f"be2_{l}", (128, DT), dr["be2r"][l]))
            encw1_sb = cload("encw1_sb", (128, 32, 128),
                             dr["encw1"].rearrange("(a p) m -> p a m", p=128))
            encb1_sb = cload("encb1_sb", (128, 1), dr["encb1"][:, :])
            encw2_sb = cload("encw2_sb", (128, 2), dr["encw2"][:, :])
            encb2n_sb = cload("encb2n_sb", (2, 1), dr["encb2n"][:, :])
            decw1_sb = cload("decw1_sb", (2, 128), dr["decw1"][:, :])
            decb1_sb = cload("decb1_sb", (128, 1), dr["decb1"][:, :])
            decw2_sb = cload("decw2_sb", (128, 32, 128),
                             dr["decw2"].rearrange("p (a m) -> p a m", m=128))
            decb2bc_sb = [cload(f"decb2bc_{par}", (128, 512), dr["decb2bc"][par]) for par in range(2)]

            oneC = const.tile([128, 128], F32, name="oneC")
            nc.vector.memset(oneC[:], 1.0 / D)
            ones32 = const.tile([128, 32], F32, name="ones32")
            nc.vector.memset(ones32[:], 1.0)
            zero128 = const.tile([128, 1], F32, name="zero128")
            nc.vector.memset(zero128[:], 0.0)
            eps128 = const.tile([128, 1], F32, name="eps128")
            nc.vector.memset(eps128[:], EPS)
            zero2 = const.tile([2, 1], F32, name="zero2")
            nc.vector.memset(zero2[:], 0.0)
            one2 = const.tile([2, 1], F32, name="one2")
            nc.vector.memset(one2[:], 1.0)

            # ---- embedding: z0T = (xt_aug^T @ embw_aug)^T ----
            z = []
            for mt in range(DT):
                ps = psG.tile([128, 512], F32, name="ps_emb", tag="g")
                nc.tensor.matmul(ps[:], embw_sb[:, mt * 128:(mt + 1) * 128], xt_sb[:],
                                 start=True, stop=True)
                t = act.tile([128, 512], F32, name=f"z_emb{mt}", tag="z", bufs=3)
                nc.vector.tensor_copy(t[:], ps[:])
                z.append(t)

            def layernorm(zin, g_sb, b_sb, tag):
                ps_mu = psG.tile([128, 512], F32, name="ps_mu", tag="g")
                for kt in range(DT):
                    nc.tensor.matmul(ps_mu[:], oneC[:, :], zin[kt][:],
                                     start=(kt == 0), stop=(kt == DT - 1))
                zc = []
                for dt in range(DT):
                    t = act.tile([128, 512], F32, name=f"zc{dt}", tag="zc", bufs=3)
                    nc.vector.tensor_sub(t[:], zin[dt][:], ps_mu[:])
                    zc.append(t)
                ps_var = psG.tile([128, 512], F32, name="ps_var", tag="g")
                for dt in range(DT):
                    sq = act.tile([128, 512], F32, name=f"sq{dt}", tag="sq", bufs=2)
                    nc.vector.tensor_mul(sq[:], zc[dt][:], zc[dt][:])
                    nc.tensor.matmul(ps_var[:], oneC[:, :], sq[:],
                                     start=(dt == 0), stop=(dt == DT - 1))
                lnv = act.tile([128, 512], F32, name="lnv", tag="lnv", bufs=2)
                nc.scalar.activation(lnv[:], ps_var[:], AF.Ln, bias=eps128[:, 0:1])
                rstd = act.tile([128, 512], F32, name="rstd", tag="rstd", bufs=2)
                nc.scalar.activation(rstd[:], lnv[:], AF.Exp, bias=zero128[:, 0:1], scale=-0.5)
                out = []
                for dt in range(DT):
                    t = act.tile([128, 512], F32, name=f"ln_{tag}{dt}", tag=tag, bufs=3)
                    nc.vector.tensor_mul(t[:], zc[dt][:], rstd[:])
                    nc.vector.tensor_scalar(t[:], t[:], g_sb[:, dt:dt + 1], b_sb[:, dt:dt + 1],
                                            op0=ALU.mult, op1=ALU.add)
                    out.append(t)
                return out

            for l in range(L):
                # ---- QKV ----
                qT, kT = [], []
                for wsb, dst, nm in ((wq_sb[l], qT, "q"), (wk_sb[l], kT, "k")):
                    for mt in range(DT):
                        ps = psG.tile([128, 512], F32, name=f"ps_{nm}", tag="g")
                        for kt in range(DT):
                            nc.tensor.matmul(ps[:], wsb[kt][:, mt * 128:(mt + 1) * 128],
                                             z[kt][:], start=(kt == 0), stop=(kt == DT - 1))
                        t = act.tile([128, 512], F32, name=f"{nm}T{mt}", tag=f"{nm}T", bufs=3)
                        nc.vector.tensor_copy(t[:], ps[:])
                        dst.append(t)
                v = []
                for nt in range(4):
                    ps = psG.tile([128, 256], F32, name="ps_v", tag="g")
                    for kt in range(DT):
                        nc.tensor.matmul(ps[:], z[kt][:, nt * 128:(nt + 1) * 128],
                                         wv_sb[l][kt][:], start=(kt == 0), stop=(kt == DT - 1))
                    t = act.tile([128, 256], F32, name=f"v{nt}", tag="v", bufs=5)
                    nc.vector.tensor_copy(t[:], ps[:])
                    v.append(t)

                # ---- attention (transposed scores, folded softmax denom) ----
                oT = []
                for g in range(2):
                    ps_o = psO.tile([128, 512], F32, name="ps_o", tag="o")
                    ps_d = psD.tile([128, 512], F32, name="ps_d", tag="d")
                    for hp in range(4):
                        h = g * 4 + hp
                        po = hp * 32
                        exs = []
                        for half in range(2):
                            ps_s = psS.tile([128, 1024], F32, name="ps_s", tag="s")
                            for j in range(2):
                                kt = half * 2 + j
                                nc.tensor.matmul(
                                    ps_s[:, j * 512:(j + 1) * 512],
                                    kT[g][po:po + 32, kt * 128:(kt + 1) * 128],
                                    qT[g][po:po + 32, :], start=True, stop=True,
                                    tile_position=(po, 0))
                            ex = act.tile([128, 1024], F32, name="ex", tag="exp", bufs=3)
                            nc.scalar.activation(ex[:], ps_s[:], AF.Exp, bias=zero128[:, 0:1], scale=SCALE)
                            exs.append(ex)
                        for kt in range(4):
                            exj = exs[kt // 2][:, (kt % 2) * 512:(kt % 2 + 1) * 512]
                            nc.tensor.matmul(ps_d[po:po + 32, :], ones32[:, :], exj,
                                             start=(kt == 0), stop=(kt == 3),
                                             tile_position=(0, po))
                        for kt in range(4):
                            exj = exs[kt // 2][:, (kt % 2) * 512:(kt % 2 + 1) * 512]
                            nc.tensor.matmul(ps_o[po:po + 32, :],
                                             v[kt][:, h * 32:(h + 1) * 32], exj,
                                             start=(kt == 0), stop=(kt == 3),
                                             tile_position=(0, po))
                    lnd = act.tile([128, 512], F32, name="lnd", tag="lnd", bufs=2)
                    nc.scalar.activation(lnd[:], ps_d[:], AF.Ln, bias=zero128[:, 0:1])
                    dinv = act.tile([128, 512], F32, name="dinv", tag="dinv", bufs=2)
                    nc.scalar.activation(dinv[:], lnd[:], AF.Exp, bias=zero128[:, 0:1], scale=-1.0)
                    ot = act.tile([128, 512], F32, name=f"oT{g}", tag="oT", bufs=3)
                    nc.vector.tensor_mul(ot[:], ps_o[:], dinv[:])
                    oT.append(ot)

                # ---- Wo projection + residual ----
                z1 = []
                for mt in range(DT):
                    ps = psG.tile([128, 512], F32, name="ps_wo", tag="g")
                    for kt in range(DT):
                        nc.tensor.matmul(ps[:], wo_sb[l][kt][:, mt * 128:(mt + 1) * 128],
                                         oT[kt][:], start=(kt == 0), stop=(kt == DT - 1))
                    t = act.tile([128, 512], F32, name=f"z1_{mt}", tag="z1", bufs=3)
                    nc.vector.tensor_add(t[:], ps[:], z[mt][:])
                    z1.append(t)

                zmid = layernorm(z1, g1_sb[l], be1_sb[l], "zmid")

                # ---- FFN ----
                f1 = []
                for ft in range(FT):
                    ps = psG.tile([128, 512], F32, name="ps_f1", tag="g")
                    for kt in range(DT):
                        nc.tensor.matmul(ps[:], w1_sb[l][kt][:, ft * 128:(ft + 1) * 128],
                                         zmid[kt][:], start=(kt == 0), stop=(kt == DT - 1))
                    t = act.tile([128, 512], F32, name=f"f1_{ft}", tag="f1", bufs=9)
                    if ft % 2 == 0:
                        nc.vector.tensor_scalar(t[:], ps[:], b1_sb[l][:, ft:ft + 1], 0.0,
                                                op0=ALU.add, op1=ALU.max)
                    else:
                        nc.scalar.activation(t[:], ps[:], AF.Relu, bias=b1_sb[l][:, ft:ft + 1])
                    f1.append(t)
                z2 = []
                for dt in range(DT):
                    ps = psG.tile([128, 512], F32, name="ps_w2", tag="g")
                    for ft in range(FT):
                        nc.tensor.matmul(ps[:], w2_sb[l][ft][:, dt * 128:(dt + 1) * 128],
                                         f1[ft][:], start=(ft == 0), stop=(ft == FT - 1))
                    t = act.tile([128, 512], F32, name=f"z2_{dt}", tag="z2", bufs=3)
                    nc.vector.tensor_add(t[:], ps[:], zmid[dt][:])
                    nc.vector.tensor_scalar(t[:], t[:], b2_sb[l][:, dt:dt + 1], None, op0=ALU.add)
                    z2.append(t)

                z = layernorm(z2, g2_sb[l], be2_sb[l], "z")

            # ---- autoencoder ----
            ps_a = psG.tile([128, 32], F32, name="ps_a", tag="g")
            for kt in range(32):
                ztv = z[kt % 2].rearrange("p (s t) -> p t s", t=T)[:, kt // 2, :]
                nc.tensor.matmul(ps_a[:], encw1_sb[:, kt, :], ztv,
                                 start=(kt == 0), stop=(kt == 31))
            a1 = act.tile([128, 32], F32, name="a1", tag="ae")
            nc.vector.tensor_scalar(a1[:], ps_a[:], encb1_sb[:, 0:1], 0.0,
                                    op0=ALU.add, op1=ALU.max)
            ps_low = psG.tile([2, 32], F32, name="ps_low", tag="g")
            nc.tensor.matmul(ps_low[:], encw2_sb[:], a1[:], start=True, stop=True)
            # sigmoid(x + b) = exp(-ln(1 + exp(-(x + b))))
            e0 = act.tile([2, 32], F32, name="e0", tag="ae2")
            nc.scalar.activation(e0[:], ps_low[:], AF.Exp, bias=encb2n_sb[:, 0:1], scale=-1.0)
            e1 = act.tile([2, 32], F32, name="e1", tag="ae2")
            nc.scalar.activation(e1[:], e0[:], AF.Ln, bias=one2[:, 0:1])
            lowt_sb = act.tile([2, 32], F32, name="lowt_sb", tag="ae2")
            nc.scalar.activation(lowt_sb[:], e1[:], AF.Exp, bias=zero2[:, 0:1], scale=-1.0)
            nc.sync.dma_start(out_low[:, :], lowt_sb[:])

            ps_d1 = psG.tile([128, 32], F32, name="ps_d1", tag="g")
            nc.tensor.matmul(ps_d1[:], decw1_sb[:], lowt_sb[:], start=True, stop=True)
            d1 = act.tile([128, 32], F32, name="d1", tag="ae")
            nc.vector.tensor_scalar(d1[:], ps_d1[:], decb1_sb[:, 0:1], 0.0,
                                    op0=ALU.add, op1=ALU.max)

            ps_rec = [psG.tile([128, 512], F32, name=f"ps_rec{par}", tag="g")
                      for par in range(2)]
            for mt in range(32):
                par = mt % 2
                tloc = mt // 2
                nc.tensor.matmul(ps_rec[par][:, tloc * 32:(tloc + 1) * 32],
                                 decw2_sb[:, mt, :], d1[:], start=True, stop=True)
            l1sb = act.tile([128, 2], F32, name="l1sb", tag="l1")
            for par in range(2):
                ztv = z[par].rearrange("p (s t) -> p t s", t=T)
                psv = ps_rec[par].rearrange("p (t s) -> p t s", s=S)
                diff = act.tile([128, 16, 32], F32, name="diff", tag="diff", bufs=2)
                nc.vector.tensor_sub(diff[:], ztv, psv)
                diff2 = act.tile([128, 512], F32, name="diff2", tag="diff2", bufs=2)
                nc.vector.tensor_sub(diff2[:], diff.rearrange("p a b -> p (a b)"),
                                     decb2bc_sb[par][:])
                nc.vector.reduce_sum(l1sb[:, par:par + 1], diff2[:],
                                     axis=mybir.AxisListType.X, apply_absolute_value=True)
            nc.sync.dma_start(out_l1[:, :], l1sb[:])

    nc.compile()
    return nc


def _prep_inputs(inputs):
    f32 = np.float32
    g = {}
    g["embw"] = np.concatenate([np.asarray(inputs["emb_W"], f32),
                                np.asarray(inputs["emb_b"], f32)[None, :]], axis=0)
    for nm in ("Wq", "Wk", "Wv", "Wo"):
        g[nm.lower()] = np.ascontiguousarray(np.asarray(inputs[nm], f32))
    g["w1"] = np.ascontiguousarray(np.asarray(inputs["W1"], f32))
    g["w2"] = np.ascontiguousarray(np.asarray(inputs["W2"], f32))
    g["b1r"] = np.ascontiguousarray(np.asarray(inputs["b1"], f32).reshape(L, FT, 128).transpose(0, 2, 1))
    g["b2r"] = np.ascontiguousarray(np.asarray(inputs["b2"], f32).reshape(L, DT, 128).transpose(0, 2, 1))
    g["g1r"] = np.ascontiguousarray(np.asarray(inputs["ln1_g"], f32).reshape(L, DT, 128).transpose(0, 2, 1))
    g["be1r"] = np.ascontiguousarray(np.asarray(inputs["ln1_b"], f32).reshape(L, DT, 128).transpose(0, 2, 1))
    g["g2r"] = np.ascontiguousarray(np.asarray(inputs["ln2_g"], f32).reshape(L, DT, 128).transpose(0, 2, 1))
    g["be2r"] = np.ascontiguousarray(np.asarray(inputs["ln2_b"], f32).reshape(L, DT, 128).transpose(0, 2, 1))
    g["encw1"] = np.ascontiguousarray(np.asarray(inputs["enc_W1"], f32))
    g["encb1"] = np.asarray(inputs["enc_b1"], f32).reshape(128, 1)
    g["encw2"] = np.ascontiguousarray(np.asarray(inputs["enc_W2"], f32))
    g["encb2n"] = (-np.asarray(inputs["enc_b2"], f32)).reshape(2, 1)
    g["decw1"] = np.ascontiguousarray(np.asarray(inputs["dec_W1"], f32))
    g["decb1"] = np.asarray(inputs["dec_b1"], f32).reshape(128, 1)
    g["decw2"] = np.ascontiguousarray(np.asarray(inputs["dec_W2"], f32))
    db2 = np.asarray(inputs["dec_b2"], f32).reshape(T, 2, 128).transpose(1, 2, 0)  # [par, p, t]
    g["decb2bc"] = np.ascontiguousarray(
        np.broadcast_to(db2[:, :, :, None], (2, 128, T, S)).reshape(2, 128, 512))

    x_seg = np.asarray(inputs["x_seg"], f32)
    in_maps = []
    for b in range(B):
        xt = x_seg[b].reshape(NSEQ, F).T
        xt = np.concatenate([xt, np.ones((1, NSEQ), f32)], axis=0)
        m = dict(g)
        m["xt"] = np.ascontiguousarray(xt)
        in_maps.append(m)
    return in_maps


def _finish_host(results, inputs):
    f32 = np.float32
    low = np.empty((B * S, 2), f32)
    l1_total = f32(0.0)
    for b in range(B):
        low[b * S:(b + 1) * S, :] = results[b]["lowt"].T
        l1_total = f32(l1_total + results[b]["l1p"].sum(dtype=f32))
    rec_loss = f32(l1_total / f32(B * S * ENC_IN))

    centers = np.asarray(inputs["centers"], f32)
    N = B * S
    d2 = ((low[:, None, :] - centers[None, :, :]) ** 2).sum(-1)
    idx = np.argmin(d2, axis=1).astype(np.int32)
    arr = np.zeros(N, f32)
    arr[idx] = f32(100.0)
    m = arr.max()
    sh = arr - m
    logp = sh - f32(np.log(np.exp(sh).sum(dtype=f32)))
    logN = f32(np.log(f32(N)))
    kl = np.mean(np.exp(-logN) * (-logN - logp), dtype=f32)
    assigned = centers[idx]
    dsq = ((low - assigned) ** 2).sum(1)
    inter = dsq.mean(dtype=f32)
    sums = np.zeros(K, f32)
    np.add.at(sums, idx, dsq)
    counts = np.bincount(idx, minlength=K).astype(f32)
    nonempty = counts > 0
    means = sums / np.maximum(counts, f32(1.0))
    denom = np.maximum(nonempty.astype(f32).sum(), f32(1.0))
    intra = f32(np.where(nonempty, means, f32(0.0)).sum() / denom)
    loss = f32(inter + intra + kl + rec_loss)
    return (np.float32(loss), np.float32(kl), idx, low)


def kernel(**inputs):
    if "nc" not in _CACHE:
        _CACHE["nc"] = _build_program()
    nc = _CACHE["nc"]
    in_maps = _prep_inputs(inputs)
    res = run_bass_kernel_spmd(nc, in_maps, core_ids=list(range(B)))
    return _finish_host(res.results, inputs)


# revision 8
# speedup vs baseline: 1.5638x; 1.5638x over previous
"""Trainium2 Bass kernel for the ClusterForecasting model.

Data-parallel over batch B=8 across 8 NeuronCores (one batch element per
core): embedding -> 2 transformer layers -> autoencoder on device, tiny
cluster stage (256 points) finished on host in float32.

Device design notes:
- transposed-resident residual stream zT [D, S*T] as 2x[128,512] tiles
- all matmuls take bf16 inputs (f32 PSUM accumulation); validated offline
  to keep low/idx well within tolerance (margins ~0.19 vs errors ~2e-3)
- softmax without max-subtraction; denominators folded into the o-matmul
  via an augmented stationary [v_h | ones | zeros] (M=64), normalized via
  a 0/1 selection matmul broadcast + exp(-ln(d)) on the scalar engine
- LayerNorm partition-dim stats via all-(1/256) matmuls (replicated mean),
  rstd via exp(-0.5*ln(var+eps)); only ACT table set ever used is
  natural_log_exp_and_others
"""

import math

import numpy as np
import ml_dtypes

import concourse.bass as bass
import concourse.mybir as mybir
import concourse.tile as tile
from concourse import bacc
from concourse.bass_utils import run_bass_kernel_spmd

F32 = mybir.dt.float32
BF16 = mybir.dt.bfloat16
AF = mybir.ActivationFunctionType
ALU = mybir.AluOpType

B, S, T, F = 8, 32, 16, 8
D, H, L, FF, K = 256, 8, 2, 1024, 5
NSEQ = S * T
DH = D // H
DT = D // 128
FT = FF // 128
ENC_IN = T * D
SCALE = 1.0 / math.sqrt(DH)
EPS = 1e-5

_CACHE = {}


def _build_program(niter=1):
    nc = bacc.Bacc("TRN2", target_bir_lowering=False, debug=False)

    dr = {}

    def din(name, shape, dt=BF16):
        dr[name] = nc.dram_tensor(name, list(shape), dt, kind="ExternalInput")
        return dr[name]

    din("xt", (F + 1, NSEQ))
    din("embw", (F + 1, D))
    din("wq", (L, D, D))
    din("wk", (L, D, D))
    din("wv", (L, D, D))
    din("wopair", (L, 4, 128, D))      # [Wo_hA | 0 | Wo_hB | 0] per pair
    din("w1", (L, D, FF))
    din("b1r", (L, 128, FT), F32)
    din("w2", (L, FF, D))
    din("b2r", (L, 128, DT), F32)
    din("g1r", (L, 128, DT), F32)
    din("be1r", (L, 128, DT), F32)
    din("g2r", (L, 128, DT), F32)
    din("be2r", (L, 128, DT), F32)
    din("sel", (128, 128))             # 0/1 band-broadcast selector
    din("encw1", (ENC_IN, 128))
    din("encb1", (128, 1), F32)
    din("encw2", (128, 2))
    din("encb2n", (2, 1), F32)
    din("decw1", (2, 128))
    din("decb1", (128, 1), F32)
    din("decw2", (128, ENC_IN))
    din("decb2bc", (2, 128, 512), F32)

    out_low = nc.dram_tensor("lowt", [2, S], F32, kind="ExternalOutput")
    out_l1 = nc.dram_tensor("l1p", [128, 2], F32, kind="ExternalOutput")

    with tile.TileContext(nc) as tc:
        with (
            tc.tile_pool(name="const", bufs=1) as const,
            tc.tile_pool(name="act", bufs=2) as act,
            tc.tile_pool(name="psS", bufs=2, space="PSUM") as psS,
            tc.tile_pool(name="psP", bufs=2, space="PSUM") as psP,
            tc.tile_pool(name="psG", bufs=2, space="PSUM") as psG,
        ):
            def cload(name, shape, src, dt=BF16):
                t = const.tile(list(shape), dt, name=name)
                nc.sync.dma_start(t[:], src)
                return t

            xt_sb = cload("xt_sb", (F + 1, NSEQ), dr["xt"][:, :])
            embw_sb = cload("embw_sb", (F + 1, D), dr["embw"][:, :])
            wq_sb, wk_sb, wv_sb, wop_sb, w1_sb, w2_sb = [], [], [], [], [], []
            b1_sb, b2_sb, g1_sb, be1_sb, g2_sb, be2_sb = [], [], [], [], [], []
            for l in range(L):
                wq_sb.append([cload(f"wq{l}_{kt}", (128, D), dr["wq"][l, kt * 128:(kt + 1) * 128, :]) for kt in range(DT)])
                wk_sb.append([cload(f"wk{l}_{kt}", (128, D), dr["wk"][l, kt * 128:(kt + 1) * 128, :]) for kt in range(DT)])
                wv_sb.append([cload(f"wv{l}_{kt}", (128, D), dr["wv"][l, kt * 128:(kt + 1) * 128, :]) for kt in range(DT)])
                wop_sb.append([cload(f"wop{l}_{pp}", (128, D), dr["wopair"][l, pp]) for pp in range(4)])
                w1_sb.append([cload(f"w1{l}_{kt}", (128, FF), dr["w1"][l, kt * 128:(kt + 1) * 128, :]) for kt in range(DT)])
                w2_sb.append([cload(f"w2{l}_{ft}", (128, D), dr["w2"][l, ft * 128:(ft + 1) * 128, :]) for ft in range(FT)])
                b1_sb.append(cload(f"b1_{l}", (128, FT), dr["b1r"][l], F32))
                b2_sb.append(cload(f"b2_{l}", (128, DT), dr["b2r"][l], F32))
                g1_sb.append(cload(f"g1_{l}", (128, DT), dr["g1r"][l], F32))
                be1_sb.append(cload(f"be1_{l}", (128, DT), dr["be1r"][l], F32))
                g2_sb.append(cload(f"g2_{l}", (128, DT), dr["g2r"][l], F32))
                be2_sb.append(cload(f"be2_{l}", (128, DT), dr["be2r"][l], F32))
            sel_sb = cload("sel_sb", (128, 128), dr["sel"][:, :])
            encw1_sb = cload("encw1_sb", (128, 32, 128),
                             dr["encw1"].rearrange("(a p) m -> p a m", p=128))
            encb1_sb = cload("encb1_sb", (128, 1), dr["encb1"][:, :], F32)
            encw2_sb = cload("encw2_sb", (128, 2), dr["encw2"][:, :])
            encb2n_sb = cload("encb2n_sb", (2, 1), dr["encb2n"][:, :], F32)
            decw1_sb = cload("decw1_sb", (2, 128), dr["decw1"][:, :])
            decb1_sb = cload("decb1_sb", (128, 1), dr["decb1"][:, :], F32)
            decw2_sb = cload("decw2_sb", (128, 32, 128),
                             dr["decw2"].rearrange("p (a m) -> p a m", m=128))
            decb2bc_sb = [cload(f"decb2bc_{par}", (128, 512), dr["decb2bc"][par], F32) for par in range(2)]

            oneC = const.tile([128, 128], BF16, name="oneC")
            nc.vector.memset(oneC[:], 1.0 / D)
            zero128 = const.tile([128, 1], F32, name="zero128")
            nc.vector.memset(zero128[:], 0.0)
            eps128 = const.tile([128, 1], F32, name="eps128")
            nc.vector.memset(eps128[:], EPS)
            zero2 = const.tile([2, 1], F32, name="zero2")
            nc.vector.memset(zero2[:], 0.0)
            one2 = const.tile([2, 1], F32, name="one2")
            nc.vector.memset(one2[:], 1.0)

            import contextlib
            loop_cm = tc.For_i(0, niter) if niter > 1 else contextlib.nullcontext()
            with loop_cm:
                # ---- embedding ----
                z = []
                for mt in range(DT):
                    ps = psG.tile([128, 512], F32, name="ps_emb", tag="g")
                    nc.tensor.matmul(ps[:], embw_sb[:, mt * 128:(mt + 1) * 128], xt_sb[:],
                                     start=True, stop=True)
                    t = act.tile([128, 512], BF16, name=f"z_emb{mt}", tag="z", bufs=3)
                    nc.vector.tensor_copy(t[:], ps[:])
                    z.append(t)

                def layernorm(zin, g_sb, b_sb, tag):
                    ps_mu = psG.tile([128, 512], F32, name="ps_mu", tag="g")
                    for kt in range(DT):
                        nc.tensor.matmul(ps_mu[:], oneC[:, :], zin[kt][:],
                                         start=(kt == 0), stop=(kt == DT - 1))
                    zc = []
                    for dt in range(DT):
                        t = act.tile([128, 512], BF16, name=f"zc{dt}", tag="zc", bufs=3)
                        nc.vector.tensor_sub(t[:], zin[dt][:], ps_mu[:])
                        zc.append(t)
                    ps_var = psG.tile([128, 512], F32, name="ps_var", tag="g")
                    for dt in range(DT):
                        sq = act.tile([128, 512], BF16, name=f"sq{dt}", tag="sq", bufs=2)
                        nc.vector.tensor_mul(sq[:], zc[dt][:], zc[dt][:])
                        nc.tensor.matmul(ps_var[:], oneC[:, :], sq[:],
                                         start=(dt == 0), stop=(dt == DT - 1))
                    lnv = act.tile([128, 512], F32, name="lnv", tag="lnv", bufs=2)
                    nc.scalar.activation(lnv[:], ps_var[:], AF.Ln, bias=eps128[:, 0:1])
                    rstd = act.tile([128, 512], BF16, name="rstd", tag="rstd", bufs=2)
                    nc.scalar.activation(rstd[:], lnv[:], AF.Exp, bias=zero128[:, 0:1], scale=-0.5)
                    out = []
                    for dt in range(DT):
                        t = act.tile([128, 512], BF16, name=f"ln_{tag}{dt}", tag=tag, bufs=3)
                        nc.vector.tensor_mul(t[:], zc[dt][:], rstd[:])
                        nc.vector.tensor_scalar(t[:], t[:], g_sb[:, dt:dt + 1], b_sb[:, dt:dt + 1],
                                                op0=ALU.mult, op1=ALU.add)
                        out.append(t)
                    return out

                for l in range(L):
                    # ---- QKV ----
                    qT, kT = [], []
                    for wsb, dst, nm in ((wq_sb[l], qT, "q"), (wk_sb[l], kT, "k")):
                        for mt in range(DT):
                            ps = psG.tile([128, 512], F32, name=f"ps_{nm}", tag="g")
                            for kt in range(DT):
                                nc.tensor.matmul(ps[:], wsb[kt][:, mt * 128:(mt + 1) * 128],
                                                 z[kt][:], start=(kt == 0), stop=(kt == DT - 1))
                            t = act.tile([128, 512], BF16, name=f"{nm}T{mt}", tag=f"{nm}T", bufs=3)
                            nc.vector.tensor_copy(t[:], ps[:])
                            dst.append(t)
                    # v augmented: per k-tile [128, 8*64]: [v_h | ones | zeros(31)]
                    vaug = []
                    for nt in range(4):
                        ps = psG.tile([128, 256], F32, name="ps_v", tag="g")
                        for kt in range(DT):
                            nc.tensor.matmul(ps[:], z[kt][:, nt * 128:(nt + 1) * 128],
                                             wv_sb[l][kt][:], start=(kt == 0), stop=(kt == DT - 1))
                        t = act.tile([128, 512], BF16, name=f"vaug{nt}", tag="vaug", bufs=5)
                        nc.vector.memset(t[:], 0.0)
                        dstv = t.rearrange("p (h c) -> p h c", c=64)
                        nc.vector.tensor_copy(dstv[:, :, 0:32],
                                              ps.rearrange("p (h c) -> p h c", c=32))
                        nc.vector.memset(dstv[:, :, 32:33], 1.0)
                        vaug.append(t)

                    # ---- attention ----
                    opairs = []
                    for g in range(2):
                        for pp in range(2):
                            ps_pair = psP.tile([128, 512], F32, name="ps_pair", tag="pair")
                            for hh in range(2):
                                h = g * 4 + pp * 2 + hh
                                po = (h % 4) * 32
                                co = 64 * hh
                                for half in range(2):
                                    ps_s = psS.tile([128, 1024], F32, name="ps_s", tag="s")
                                    for j in range(2):
                                        kt = half * 2 + j
                                        nc.tensor.matmul(
                                            ps_s[:, j * 512:(j + 1) * 512],
                                            kT[g][po:po + 32, kt * 128:(kt + 1) * 128],
                                            qT[g][po:po + 32, :], start=True, stop=True,
                                            tile_position=(po, 0))
                                    ex = act.tile([128, 1024], BF16, name="ex", tag="exp", bufs=3)
                                    nc.scalar.activation(ex[:], ps_s[:], AF.Exp,
                                                         bias=zero128[:, 0:1], scale=SCALE)
                                    for j in range(2):
                                        kt = half * 2 + j
                                        nc.tensor.matmul(
                                            ps_pair[co:co + 64, :],
                                            vaug[kt][:, h * 64:(h + 1) * 64],
                                            ex[:, j * 512:(j + 1) * 512],
                                            start=(kt == 0), stop=(kt == 3),
                                            tile_position=(0, co))
                            pr = act.tile([128, 512], BF16, name="pr", tag="pr", bufs=2)
                            nc.vector.tensor_copy(pr[:], ps_pair[:])
                            ps_db = psG.tile([128, 512], F32, name="ps_db", tag="g")
                            nc.tensor.matmul(ps_db[:], sel_sb[:, :], pr[:], start=True, stop=True)
                            lnd = act.tile([128, 512], F32, name="lnd", tag="lnd", bufs=2)
                            nc.scalar.activation(lnd[:], ps_db[:], AF.Ln, bias=zero128[:, 0:1])
                            dinv = act.tile([128, 512], BF16, name="dinv", tag="dinv", bufs=2)
                            nc.scalar.activation(dinv[:], lnd[:], AF.Exp,
                                                 bias=zero128[:, 0:1], scale=-1.0)
                            op = act.tile([128, 512], BF16, name="opair", tag="opair", bufs=5)
                            nc.vector.tensor_mul(op[:], pr[:], dinv[:])
                            opairs.append(op)

                    # ---- Wo + residual ----
                    z1 = []
                    for mt in range(DT):
                        ps = psG.tile([128, 512], F32, name="ps_wo", tag="g")
                        for ppi in range(4):
                            nc.tensor.matmul(ps[:], wop_sb[l][ppi][:, mt * 128:(mt + 1) * 128],
                                             opairs[ppi][:], start=(ppi == 0), stop=(ppi == 3))
                        t = act.tile([128, 512], BF16, name=f"z1_{mt}", tag="z1", bufs=3)
                        nc.vector.tensor_add(t[:], ps[:], z[mt][:])
                        z1.append(t)

                    zmid = layernorm(z1, g1_sb[l], be1_sb[l], "zmid")

                    # ---- FFN ----
                    f1 = []
                    for ft in range(FT):
                        ps = psG.tile([128, 512], F32, name="ps_f1", tag="g")
                        for kt in range(DT):
                            nc.tensor.matmul(ps[:], w1_sb[l][kt][:, ft * 128:(ft + 1) * 128],
                                             zmid[kt][:], start=(kt == 0), stop=(kt == DT - 1))
                        t = act.tile([128, 512], BF16, name=f"f1_{ft}", tag="f1", bufs=9)
                        if ft % 2 == 0:
                            nc.vector.tensor_scalar(t[:], ps[:], b1_sb[l][:, ft:ft + 1], 0.0,
                                                    op0=ALU.add, op1=ALU.max)
                        else:
                            nc.scalar.activation(t[:], ps[:], AF.Relu, bias=b1_sb[l][:, ft:ft + 1])
                        f1.append(t)
                    z2 = []
                    for dt in range(DT):
                        ps = psG.tile([128, 512], F32, name="ps_w2", tag="g")
                        for ft in range(FT):
                            nc.tensor.matmul(ps[:], w2_sb[l][ft][:, dt * 128:(dt + 1) * 128],
                                             f1[ft][:], start=(ft == 0), stop=(ft == FT - 1))
                        t = act.tile([128, 512], BF16, name=f"z2_{dt}", tag="z2", bufs=3)
                        nc.vector.tensor_add(t[:], ps[:], zmid[dt][:])
                        nc.vector.tensor_scalar(t[:], t[:], b2_sb[l][:, dt:dt + 1], None, op0=ALU.add)
                        z2.append(t)

                    z = layernorm(z2, g2_sb[l], be2_sb[l], "z")

                # ---- autoencoder ----
                ps_a = psG.tile([128, 32], F32, name="ps_a", tag="g")
                for kt in range(32):
                    ztv = z[kt % 2].rearrange("p (s t) -> p t s", t=T)[:, kt // 2, :]
                    nc.tensor.matmul(ps_a[:], encw1_sb[:, kt, :], ztv,
                                     start=(kt == 0), stop=(kt == 31))
                a1 = act.tile([128, 32], BF16, name="a1", tag="ae")
                nc.vector.tensor_scalar(a1[:], ps_a[:], encb1_sb[:, 0:1], 0.0,
                                        op0=ALU.add, op1=ALU.max)
                ps_low = psG.tile([2, 32], F32, name="ps_low", tag="g")
                nc.tensor.matmul(ps_low[:], encw2_sb[:], a1[:], start=True, stop=True)
                e0 = act.tile([2, 32], F32, name="e0", tag="ae2")
                nc.scalar.activation(e0[:], ps_low[:], AF.Exp, bias=encb2n_sb[:, 0:1], scale=-1.0)
                e1 = act.tile([2, 32], F32, name="e1", tag="ae2")
                nc.scalar.activation(e1[:], e0[:], AF.Ln, bias=one2[:, 0:1])
                lowt_sb = act.tile([2, 32], F32, name="lowt_sb", tag="ae2")
                nc.scalar.activation(lowt_sb[:], e1[:], AF.Exp, bias=zero2[:, 0:1], scale=-1.0)
                nc.sync.dma_start(out_low[:, :], lowt_sb[:])
                lowt_bf = act.tile([2, 32], BF16, name="lowt_bf", tag="ae2b")
                nc.vector.tensor_copy(lowt_bf[:], lowt_sb[:])

                ps_d1 = psG.tile([128, 32], F32, name="ps_d1", tag="g")
                nc.tensor.matmul(ps_d1[:], decw1_sb[:], lowt_bf[:], start=True, stop=True)
                d1 = act.tile([128, 32], BF16, name="d1", tag="ae")
                nc.vector.tensor_scalar(d1[:], ps_d1[:], decb1_sb[:, 0:1], 0.0,
                                        op0=ALU.add, op1=ALU.max)

                ps_rec = [psG.tile([128, 512], F32, name=f"ps_rec{par}", tag="g")
                          for par in range(2)]
                for mt in range(32):
                    par = mt % 2
                    tloc = mt // 2
                    nc.tensor.matmul(ps_rec[par][:, tloc * 32:(tloc + 1) * 32],
                                     decw2_sb[:, mt, :], d1[:], start=True, stop=True)
                l1sb = act.tile([128, 2], F32, name="l1sb", tag="l1")
                for par in range(2):
                    ztv = z[par].rearrange("p (s t) -> p t s", t=T)
                    psv = ps_rec[par].rearrange("p (t s) -> p t s", s=S)
                    diff = act.tile([128, 16, 32], F32, name="diff", tag="diff", bufs=2)
                    nc.vector.tensor_sub(diff[:], ztv, psv)
                    diff2 = act.tile([128, 512], F32, name="diff2", tag="diff2", bufs=2)
                    nc.vector.tensor_sub(diff2[:], diff.rearrange("p a b -> p (a b)"),
                                         decb2bc_sb[par][:])
                    nc.vector.reduce_sum(l1sb[:, par:par + 1], diff2[:],
                                         axis=mybir.AxisListType.X, apply_absolute_value=True)
                nc.sync.dma_start(out_l1[:, :], l1sb[:])

    nc.compile()
    return nc


def _prep_inputs(inputs):
    f32 = np.float32
    bf16 = ml_dtypes.bfloat16
    g = {}
    g["embw"] = np.concatenate([np.asarray(inputs["emb_W"], f32),
                                np.asarray(inputs["emb_b"], f32)[None, :]], axis=0).astype(bf16)
    for nm in ("Wq", "Wk", "Wv"):
        g[nm.lower()] = np.asarray(inputs[nm], f32).astype(bf16)
    wo = np.asarray(inputs["Wo"], f32)
    wopair = np.zeros((L, 4, 128, D), f32)
    for pp in range(4):
        hA, hB = 2 * pp, 2 * pp + 1
        wopair[:, pp, 0:32, :] = wo[:, hA * 32:(hA + 1) * 32, :]
        wopair[:, pp, 64:96, :] = wo[:, hB * 32:(hB + 1) * 32, :]
    g["wopair"] = wopair.astype(bf16)
    sel = np.zeros((128, 128), f32)
    sel[32, 0:64] = 1.0
    sel[96, 64:128] = 1.0
    g["sel"] = sel.astype(bf16)
    g["w1"] = np.asarray(inputs["W1"], f32).astype(bf16)
    g["w2"] = np.asarray(inputs["W2"], f32).astype(bf16)
    g["b1r"] = np.asarray(inputs["b1"], f32).reshape(L, FT, 128).transpose(0, 2, 1)
    g["b2r"] = np.asarray(inputs["b2"], f32).reshape(L, DT, 128).transpose(0, 2, 1)
    g["g1r"] = np.asarray(inputs["ln1_g"], f32).reshape(L, DT, 128).transpose(0, 2, 1)
    g["be1r"] = np.asarray(inputs["ln1_b"], f32).reshape(L, DT, 128).transpose(0, 2, 1)
    g["g2r"] = np.asarray(inputs["ln2_g"], f32).reshape(L, DT, 128).transpose(0, 2, 1)
    g["be2r"] = np.asarray(inputs["ln2_b"], f32).reshape(L, DT, 128).transpose(0, 2, 1)
    g["encw1"] = np.asarray(inputs["enc_W1"], f32).astype(bf16)
    g["encb1"] = np.asarray(inputs["enc_b1"], f32).reshape(128, 1)
    g["encw2"] = np.asarray(inputs["enc_W2"], f32).astype(bf16)
    g["encb2n"] = (-np.asarray(inputs["enc_b2"], f32)).reshape(2, 1)
    g["decw1"] = np.asarray(inputs["dec_W1"], f32).astype(bf16)
    g["decb1"] = np.asarray(inputs["dec_b1"], f32).reshape(128, 1)
    g["decw2"] = np.asarray(inputs["dec_W2"], f32).astype(bf16)
    db2 = np.asarray(inputs["dec_b2"], f32).reshape(T, 2, 128).transpose(1, 2, 0)
    g["decb2bc"] = np.ascontiguousarray(
        np.broadcast_to(db2[:, :, :, None], (2, 128, T, S)).reshape(2, 128, 512))

    x_seg = np.asarray(inputs["x_seg"], f32)
    in_maps = []
    for b in range(B):
        xt = x_seg[b].reshape(NSEQ, F).T
        xt = np.concatenate([xt, np.ones((1, NSEQ), f32)], axis=0)
        m = dict(g)
        m["xt"] = np.ascontiguousarray(xt).astype(bf16)
        in_maps.append(m)
    return in_maps


def _finish_host(results, inputs):
    f32 = np.float32
    low = np.empty((B * S, 2), f32)
    l1_total = f32(0.0)
    for b in range(B):
        low[b * S:(b + 1) * S, :] = results[b]["lowt"].T
        l1_total = f32(l1_total + results[b]["l1p"].sum(dtype=f32))
    rec_loss = f32(l1_total / f32(B * S * ENC_IN))

    centers = np.asarray(inputs["centers"], f32)
    N = B * S
    d2 = ((low[:, None, :] - centers[None, :, :]) ** 2).sum(-1)
    idx = np.argmin(d2, axis=1).astype(np.int32)
    arr = np.zeros(N, f32)
    arr[idx] = f32(100.0)
    sh = arr - arr.max()
    logp = sh - f32(np.log(np.exp(sh).sum(dtype=f32)))
    logN = f32(np.log(f32(N)))
    kl = np.mean(np.exp(-logN) * (-logN - logp), dtype=f32)
    assigned = centers[idx]
    dsq = ((low - assigned) ** 2).sum(1)
    inter = dsq.mean(dtype=f32)
    sums = np.zeros(K, f32)
    np.add.at(sums, idx, dsq)
    counts = np.bincount(idx, minlength=K).astype(f32)
    nonempty = counts > 0
    means = sums / np.maximum(counts, f32(1.0))
    denom = np.maximum(nonempty.astype(f32).sum(), f32(1.0))
    intra = f32(np.where(nonempty, means, f32(0.0)).sum() / denom)
    loss = f32(inter + intra + kl + rec_loss)
    return (np.float32(loss), np.float32(kl), idx, low)


def kernel(**inputs):
    if "nc" not in _CACHE:
        _CACHE["nc"] = _build_program()
    nc = _CACHE["nc"]
    in_maps = _prep_inputs(inputs)
    res = run_bass_kernel_spmd(nc, in_maps, core_ids=list(range(B)))
    return _finish_host(res.results, inputs)
